# revision 1
# baseline (speedup 1.0000x reference)
"""Covariance pooling kernel for Trainium2 (8 NeuronCores, data-parallel over batch).

y[b] = (1/M) * (x[b] - mean(x[b])) @ (x[b] - mean(x[b]))^T  with x[b] [C=128, M=4096].

The kernel is HBM-read bound: 16.78 MB fp32 per core (~52 us at the achieved
~330 GB/s/core). Everything else hides under the stream:
  - one SWDGE cast DMA per batch (fp32 HBM -> fp8_e4m3 SBUF): 16 KB reads per
    descriptor row stream at peak rate and are contention-immune; the first
    and last batches are split 4x (first so the PE starts on the earliest
    quarter, last so the post-stream tail is one chunk-group, not a batch)
  - all 8 fp8 batches stay resident in SBUF (32 KB/partition) so every DMA is
    enqueued up front with no buffer-reuse waits anywhere in the stream
  - per 256-spatial pair: two fp8 PE transposes (mandatory element-step-2
    PSUM writes at even offsets), one DVE/ACT copy interleaves the pair into
    an SBUF slot, one DoubleRowSwInterleave matmul (K=256) accumulates the
    gram; a constant ones column makes the same matmul produce row sums
  - DoubleRowSwInterleave reads stationary columns reversed, so PSUM ends as
    [P@G | P@s] (rows flipped); the device only scales by 1/M and writes 129
    columns -- the host un-flips rows and applies the rank-1 mean correction
    (0.005% of the FLOPs) while gathering shards
"""

import numpy as np

import ml_dtypes
import concourse.bass as bass
import concourse.tile as tile
from concourse import bacc, mybir
from concourse.bass_utils import run_bass_kernel_spmd

N_CORES = 8
B_FULL = 64
B_CORE = B_FULL // N_CORES  # 8 batches per core
C = 128
M = 4096  # 64*64 spatial
PAIRS = M // 256  # 16 chunk pairs per batch
NSLOT = 8  # SBUF pair-slot ring
F32 = mybir.dt.float32
FP8 = mybir.dt.float8e4
COPY = mybir.ActivationFunctionType.Copy
DRSW = mybir.MatmulPerfMode.DoubleRowSwInterleave

_CACHE: dict = {}


def _build_program() -> bass.Bass:
    nc = bacc.Bacc()
    x = nc.declare_dram_parameter("x", [B_CORE, C, M], F32, isOutput=False)
    ident8 = nc.declare_dram_parameter("ident8", [C, C], FP8, isOutput=False)
    y = nc.declare_dram_parameter("y", [B_CORE, C, 129], F32, isOutput=True)

    with tile.TileContext(nc) as tc:
        with (
            tc.tile_pool(name="singles", bufs=1) as singles,
            tc.tile_pool(name="yout", bufs=3) as yout_pool,
            tc.tile_pool(name="tp", bufs=4, space="PSUM") as tp_pool,
            tc.tile_pool(name="gram", bufs=3, space="PSUM") as gram_pool,
        ):
            # whole input, fp8, resident: DMAs enqueue back-to-back with no
            # reuse hazards; 16 KB-per-row descriptors for peak throughput
            xb = singles.tile([C, B_CORE, M], FP8)
            splits = {0: 4, B_CORE - 1: 8}  # early PE start / short tail
            for b in range(B_CORE):
                n = splits.get(b, 1)
                step = M // n
                for h in range(n):
                    nc.gpsimd.dma_start(
                        xb[:, b, h * step : (h + 1) * step],
                        x[b][:, h * step : (h + 1) * step],
                    )

            identity8 = singles.tile([C, C], FP8)
            nc.sync.dma_start(identity8, ident8[:, :])

            # pair slots: fp8 byte 2c+t = chunk t col c; col 128 = ones
            # (feeds row sums through the gram matmul), col 129 = zero pad
            xt = singles.tile([C, NSLOT, 130, 2], FP8)
            nc.vector.memset(xt[:, :, 128, :], 1.0)
            nc.vector.memset(xt[:, :, 129, :], 0.0)

            # PE warm-up absorbs the identity-DMA wait before data arrives
            warm = tp_pool.tile([C, 2, C, 2], FP8, tag="tp")
            nc.tensor.transpose(warm[:, 0, :, 0], identity8, identity8)

            for b in range(B_CORE):
                gram = gram_pool.tile([C, 130], F32)
                for g in range(PAIRS):
                    tp = tp_pool.tile([C, 2, C, 2], FP8, tag="tp")
                    for j in range(2):
                        k = 2 * g + j
                        nc.tensor.transpose(
                            tp[:, j, :, 0],
                            xb[:, b, k * 128 : (k + 1) * 128],
                            identity8,
                        )
                    s = g % NSLOT
                    # dense-write orientation: out iterates c-major (bytes
                    # sequential), in reads strided across the two regions
                    dst = xt[:, s, 0:128, :]
                    src = tp[:, :, :, 0].rearrange("p t c -> p c t")
                    if g % 2 == 0:
                        nc.vector.tensor_copy(dst, src)
                    else:
                        nc.scalar.activation(dst, src, COPY)
                    nc.tensor.matmul(
                        gram,
                        xt[:, s, 0:128, :],
                        xt[:, s, 0:130, :].rearrange("p c t -> p t c"),
                        start=(g == 0),
                        stop=(g == PAIRS - 1),
                        perf_mode=DRSW,
                    )

                y_tile = yout_pool.tile([C, 129], F32)
                nc.vector.tensor_scalar_mul(y_tile, gram[:, 0:129], 1.0 / M)
                nc.sync.dma_start(y[b], y_tile)

    nc.compile()
    return nc


def _get_program() -> bass.Bass:
    if "nc" not in _CACHE:
        _CACHE["nc"] = _build_program()
    return _CACHE["nc"]


def _run(x: np.ndarray, **spmd_kwargs):
    x = np.ascontiguousarray(np.asarray(x), dtype=np.float32)
    assert x.shape == (B_FULL, C, 64, 64), x.shape
    xf = x.reshape(B_FULL, C, M)
    shards = np.split(xf, N_CORES, axis=0)
    ident8 = np.eye(C, dtype=ml_dtypes.float8_e4m3)
    in_maps = [{"x": s, "ident8": ident8} for s in shards]
    nc = _get_program()
    res = run_bass_kernel_spmd(nc, in_maps, list(range(N_CORES)), **spmd_kwargs)
    raw = np.concatenate([res.results[i]["y"] for i in range(N_CORES)], axis=0)
    # raw[b] = [P@G | P@s] / M (rows flipped by DoubleRowSwInterleave).
    # Un-flip and apply the rank-1 mean correction: y = G/M - (s/M)(s/M)^T
    g_flip = raw[:, ::-1, 0:128]
    sv = raw[:, ::-1, 128]  # s[c]/M, straight channel order
    out = g_flip - sv[:, :, None] * sv[:, None, :]
    return np.ascontiguousarray(out, dtype=np.float32), res


def kernel(x: np.ndarray) -> np.ndarray:
    out, _ = _run(x)
    return out

